# revision 15
# baseline (speedup 1.0000x reference)
"""Trainium2 Bass kernel for nn_CEFM_47863115547253.

Data-parallel over batch: 16 samples / 8 cores = 2 samples per core.
Per core: f0 interp (blocked payload scans) -> SG-smooth + |d2| (halo conv)
-> pair-fold to TF2 -> PE matmul against pre-transposed attention
-> per-sample energies -> AllGather(8x2) -> z-score softmax (replicated).

Self-contained: hardcodes shapes from the problem spec.
"""
import numpy as np

B, TF, TP, TF2 = 16, 8192, 512, 4096
MU_STAR, KAPPA_STAR, BETA, EPS = 350.0, 1.6, 1.0, 1e-8
NCORES = 8
NS = B // NCORES          # samples per core = 2
NB, BL = 128, 64          # time blocks (partitions) x block length
NCH = TF2 // 128          # 32 f-chunks of 128
PCH = TP // 128           # 4 phoneme chunks
LN2 = float(np.log(2.0))

_PROG_CACHE = {}


def _build_program(variant="full"):
    import concourse.bacc as bacc
    import concourse.mybir as mybir
    from concourse import tile

    F = mybir.dt.float32
    I32 = mybir.dt.int32
    A = mybir.AluOpType
    ACT = mybir.ActivationFunctionType
    AX = mybir.AxisListType

    nc = bacc.Bacc("TRN2", target_bir_lowering=False, debug=False,
                   num_devices=NCORES)

    f0_loc = nc.dram_tensor("f0_loc", [NS, TF], F, kind="ExternalInput")
    attnT_loc = nc.dram_tensor("attnT_loc", [NS, TF2, TP], F, kind="ExternalInput")
    uv_loc = nc.dram_tensor("uv_loc", [NS, TP], F, kind="ExternalInput")

    curv_out = nc.dram_tensor("curv_out", [NS, TP], F, kind="ExternalOutput")
    mean_out = nc.dram_tensor("mean_out", [NS, TP], F, kind="ExternalOutput")
    wts_out = nc.dram_tensor("wts_out", [1, B], F, kind="ExternalOutput")
    ess_out = nc.dram_tensor("ess_out", [1, 1], F, kind="ExternalOutput")

    i3_c = nc.inline_tensor(np.eye(3, dtype=np.float32), name="i3_c")
    sentF_c = nc.inline_tensor(
        np.array([[-1.0], [-1.0], [0.0], [0.0]], np.float32), name="sentF_c")
    sentB_c = nc.inline_tensor(
        np.array([[8192.0], [8192.0], [0.0], [0.0]], np.float32), name="sentB_c")

    c7 = np.array([-2.0, 3.0, 6.0, 7.0, 6.0, 3.0, -2.0], np.float64) / 21.0
    t2_np = np.broadcast_to(
        (np.arange(NB, dtype=np.float32)[:, None] * BL
         + np.arange(BL, dtype=np.float32)[None, :])[:, None, :],
        (NB, NS, BL)).copy()
    t2_c = nc.inline_tensor(t2_np, name="t2_c")
    i128_c = nc.inline_tensor(np.eye(NB, dtype=np.float32), name="i128_c")
    i4_c = nc.inline_tensor(np.eye(4, dtype=np.float32), name="i4_c")

    with tile.TileContext(nc) as tc:
        with (
            tc.tile_pool(name="fr", bufs=1) as fr,
            tc.tile_pool(name="at", bufs=8) as at,
            tc.tile_pool(name="ps", bufs=2, space="PSUM") as psp,
            tc.tile_pool(name="ps1", bufs=1, space="PSUM") as psp1,
            tc.tile_pool(name="dram", bufs=1, space="DRAM") as dr,
        ):
            # ---------------- Stage A: frame pipeline ----------------
            X = fr.tile([NB, NS, BL], F)     # f0 blocked: X[p,s,c] = f0[s, 64p+c]
            nc.sync.dma_start(X[:, :, :],
                              f0_loc[:, :].rearrange("s (p c) -> p s c", c=BL))

            T2 = fr.tile([NB, NS, BL], F)
            nc.sync.dma_start(T2[:, :, :], t2_c[:, :, :])

            U = fr.tile([NB, NS, BL], F)     # unvoiced = f0 <= 0
            nc.vector.tensor_scalar(U[:, :, :], X[:, :, :], 0.0, None, A.is_le)
            V = fr.tile([NB, NS, BL], F)     # voiced = 1 - U
            nc.vector.tensor_scalar(V[:, :, :], U[:, :, :], -1.0, 1.0, A.mult, A.add)
            VT = fr.tile([NB, NS, BL], F)    # voiced * t
            nc.vector.tensor_tensor(VT[:, :, :], V[:, :, :], T2[:, :, :], A.mult)

            # pass-1 block-local scans (state = u*state + d1), packed so the
            # block totals form one strided [128, 4] view per direction
            P1 = fr.tile([NB, 4, BL], F)   # rows: idx_s0, idx_s1, val_s0, val_s1
            B1 = fr.tile([NB, 4, BL], F)
            for s in range(NS):
                nc.vector.tensor_tensor_scan(P1[:, s, :], U[:, s, :], VT[:, s, :],
                                             -1.0, A.mult, A.add)
                nc.vector.tensor_tensor_scan(P1[:, 2 + s, :], U[:, s, :], X[:, s, :],
                                             0.0, A.mult, A.add)
                nc.vector.tensor_tensor_scan(B1[:, s, ::-1], U[:, s, ::-1],
                                             VT[:, s, ::-1], 8192.0, A.mult, A.add)
                nc.vector.tensor_tensor_scan(B1[:, 2 + s, ::-1], U[:, s, ::-1],
                                             X[:, s, ::-1], 0.0, A.mult, A.add)

            # block totals -> [4, 129] carry tiles (rows: idx_s0, idx_s1, val_s0, val_s1)
            SF = fr.tile([4, 1], F)
            SBt = fr.tile([4, 1], F)
            nc.sync.dma_start(SF[:, :], sentF_c[:, :])
            nc.sync.dma_start(SBt[:, :], sentB_c[:, :])

            I128 = fr.tile([NB, NB], F)
            nc.sync.dma_start(I128[:, :], i128_c[:, :])
            I4 = fr.tile([4, 4], F)
            nc.sync.dma_start(I4[:, :], i4_c[:, :])

            TTF = fr.tile([4, 129], F)
            TTB = fr.tile([4, 129], F)
            # fwd: col 0 neutral, col j = block j-1 total (fwd total at c=BL-1)
            # bwd: col j = block j total (bwd total at c=0), col 128 neutral
            tpF = psp.tile([4, NB], F, tag="tpc", bufs=1)
            nc.tensor.transpose(tpF[:, :], P1[:, :, BL - 1], I128[:, :])
            nc.vector.tensor_copy(TTF[:, 1:129], tpF[:, :])
            tpB = psp.tile([4, NB], F, tag="tpc", bufs=1)
            nc.tensor.transpose(tpB[:, :], B1[:, :, 0], I128[:, :])
            nc.vector.tensor_copy(TTB[:, 0:128], tpB[:, :])
            nc.vector.tensor_copy(TTF[:, 0:1], SF[:, :])
            nc.vector.tensor_copy(TTB[:, 128:129], SBt[:, :])

            UUF = fr.tile([4, 129], F)
            NNF = fr.tile([4, 129], F)
            D1F = fr.tile([4, 129], F)
            CYF = fr.tile([4, 129], F)
            nc.vector.tensor_scalar(UUF[:, :], TTF[:, :], SF[:, 0:1], None, A.is_equal)
            nc.vector.tensor_scalar(NNF[:, :], UUF[:, :], -1.0, 1.0, A.mult, A.add)
            nc.vector.tensor_tensor(D1F[:, :], TTF[:, :], NNF[:, :], A.mult)
            nc.vector.tensor_tensor_scan(CYF[:, :], UUF[:, :], D1F[:, :],
                                         SF[:, 0:1], A.mult, A.add)

            UUB = fr.tile([4, 129], F)
            NNB = fr.tile([4, 129], F)
            D1B = fr.tile([4, 129], F)
            CYB = fr.tile([4, 129], F)
            nc.vector.tensor_scalar(UUB[:, :], TTB[:, :], SBt[:, 0:1], None, A.is_equal)
            nc.vector.tensor_scalar(NNB[:, :], UUB[:, :], -1.0, 1.0, A.mult, A.add)
            nc.vector.tensor_tensor(D1B[:, :], TTB[:, :], NNB[:, :], A.mult)
            nc.vector.tensor_tensor_scan(CYB[:, ::-1], UUB[:, ::-1], D1B[:, ::-1],
                                         SBt[:, 0:1], A.mult, A.add)

            # carries back to [128, 4] (cols: idx_s0, idx_s1, val_s0, val_s1)
            FC = fr.tile([NB, 4], F)
            BC = fr.tile([NB, 4], F)
            tpFC = psp.tile([NB, 4], F, tag="tpc2", bufs=1)
            nc.tensor.transpose(tpFC[:, :], CYF[:, 0:128], I4[:, :])
            nc.vector.tensor_copy(FC[:, :], tpFC[:, :])
            tpBC = psp.tile([NB, 4], F, tag="tpc2", bufs=1)
            nc.tensor.transpose(tpBC[:, :], CYB[:, 1:129], I4[:, :])
            nc.vector.tensor_copy(BC[:, :], tpBC[:, :])

            # pass-2 global scans
            P2I = fr.tile([NB, NS, BL], F)
            P2V = fr.tile([NB, NS, BL], F)
            B2I = fr.tile([NB, NS, BL], F)
            B2V = fr.tile([NB, NS, BL], F)
            for s in range(NS):
                nc.vector.tensor_tensor_scan(P2I[:, s, :], U[:, s, :], VT[:, s, :],
                                             FC[:, s:s + 1], A.mult, A.add)
                nc.vector.tensor_tensor_scan(P2V[:, s, :], U[:, s, :], X[:, s, :],
                                             FC[:, 2 + s:3 + s], A.mult, A.add)
                nc.vector.tensor_tensor_scan(B2I[:, s, ::-1], U[:, s, ::-1],
                                             VT[:, s, ::-1], BC[:, s:s + 1],
                                             A.mult, A.add)
                nc.vector.tensor_tensor_scan(B2V[:, s, ::-1], U[:, s, ::-1],
                                             X[:, s, ::-1], BC[:, 2 + s:3 + s],
                                             A.mult, A.add)

            # blend + linear interp (masks must be integer dtype for copy_predicated)
            U32 = mybir.dt.uint32
            MP = fr.tile([NB, NS, BL], U32)
            MN = fr.tile([NB, NS, BL], U32)
            VMu = fr.tile([NB, NS, BL], U32)
            nc.vector.tensor_scalar(MP[:, :, :], P2I[:, :, :], 0.0, None, A.is_ge)
            nc.vector.tensor_scalar(MN[:, :, :], B2I[:, :, :], float(TF), None, A.is_lt)
            nc.vector.tensor_scalar(VMu[:, :, :], X[:, :, :], 0.0, None, A.is_gt)
            LT = fr.tile([NB, NS, BL], F)
            LV = fr.tile([NB, NS, BL], F)
            HT = fr.tile([NB, NS, BL], F)
            HV = fr.tile([NB, NS, BL], F)
            for s in range(NS):
                nc.vector.select(LT[:, s, :], MP[:, s, :], P2I[:, s, :], B2I[:, s, :])
                nc.vector.select(LV[:, s, :], MP[:, s, :], P2V[:, s, :], B2V[:, s, :])
                nc.vector.select(HT[:, s, :], MN[:, s, :], B2I[:, s, :], P2I[:, s, :])
                nc.vector.select(HV[:, s, :], MN[:, s, :], B2V[:, s, :], P2V[:, s, :])

            SPAN = fr.tile([NB, NS, BL], F)
            nc.vector.tensor_tensor(SPAN[:, :, :], HT[:, :, :], LT[:, :, :], A.subtract)
            nc.vector.tensor_scalar(SPAN[:, :, :], SPAN[:, :, :], 1.0, None, A.max)
            RS = fr.tile([NB, NS, BL], F)
            nc.vector.reciprocal(RS[:, :, :], SPAN[:, :, :])
            TD = fr.tile([NB, NS, BL], F)
            nc.vector.tensor_tensor(TD[:, :, :], T2[:, :, :], LT[:, :, :], A.subtract)
            W = fr.tile([NB, NS, BL], F)
            nc.vector.tensor_tensor(W[:, :, :], TD[:, :, :], RS[:, :, :], A.mult)
            nc.vector.tensor_scalar(W[:, :, :], W[:, :, :], 0.0, 1.0, A.max, A.min)
            DV = fr.tile([NB, NS, BL], F)
            nc.vector.tensor_tensor(DV[:, :, :], HV[:, :, :], LV[:, :, :], A.subtract)
            XW = fr.tile([NB, NS, BL], F)
            nc.vector.tensor_tensor(XW[:, :, :], W[:, :, :], DV[:, :, :], A.mult)
            nc.vector.tensor_tensor(XW[:, :, :], XW[:, :, :], LV[:, :, :], A.add)

            # interp result straight into the halo tile interior (cols 4..68)
            XIH = fr.tile([NB, NS, 72], F)
            for s in range(NS):
                nc.vector.select(XIH[:, s, 4:68], VMu[:, s, :], X[:, s, :], XW[:, s, :])

            # halos: cross-block from neighbors, replicate at global edges
            nc.sync.dma_start(XIH[1:NB, :, 0:4], XIH[0:NB - 1, :, 64:68])
            nc.sync.dma_start(XIH[0:NB - 1, :, 68:72], XIH[1:NB, :, 4:8])
            nc.vector.tensor_copy(XIH[0:1, :, 0:4],
                                  XIH[0:1, :, 4:5].to_broadcast([1, NS, 4]))
            STG = fr.tile([1, NS, 5], F)
            nc.sync.dma_start(STG[0:1, :, 0:1], XIH[NB - 1:NB, :, 67:68])
            nc.vector.tensor_copy(STG[0:1, :, 1:5],
                                  STG[0:1, :, 0:1].to_broadcast([1, NS, 4]))
            nc.sync.dma_start(XIH[NB - 1:NB, :, 68:72], STG[0:1, :, 1:5])

            # SG smooth (7 taps) -> Y[p, s, j] = y(t = 64p + j - 1), j in 0..65
            Y = fr.tile([NB, NS, 66], F)
            nc.vector.tensor_scalar(Y[:, :, :], XIH[:, :, 0:66], float(c7[0]), None,
                                    A.mult)
            for k in range(1, 7):
                nc.vector.scalar_tensor_tensor(Y[:, :, :], XIH[:, :, k:k + 66],
                                               float(c7[k]), Y[:, :, :],
                                               A.mult, A.add)
            # replicate-pad y at the global edges
            nc.vector.tensor_copy(Y[0:1, :, 0:1], Y[0:1, :, 1:2])
            nc.sync.dma_start(Y[NB - 1:NB, :, 65:66], Y[NB - 1:NB, :, 64:65])

            # |second derivative| and pair-fold
            D2 = fr.tile([NB, NS, BL], F)
            nc.vector.tensor_tensor(D2[:, :, :], Y[:, :, 0:64], Y[:, :, 2:66], A.add)
            nc.vector.scalar_tensor_tensor(D2[:, :, :], Y[:, :, 1:65], -2.0,
                                           D2[:, :, :], A.mult, A.add)
            nc.scalar.activation(D2[:, :, :], D2[:, :, :], ACT.Abs)
            D2P = fr.tile([NB, NS, 32], F)
            F0P = fr.tile([NB, NS, 32], F)
            nc.vector.tensor_tensor(D2P[:, :, :], D2[:, :, 0::2], D2[:, :, 1::2], A.add)
            nc.vector.tensor_tensor(F0P[:, :, :], X[:, :, 0::2], X[:, :, 1::2], A.add)

            # V-matrix for matmul lhsT. Contraction chunk j covers the f-set
            # {32p + j : p in 0..127} (one f per partition) — the PE sum over
            # partitions is order-invariant, so chunks need not be contiguous
            # in f. This makes lhsT a direct view of the pair tiles' column j.
            # bf16 operands: fp32 matmul streams at a fraction of bf16 rate on
            # the PE; accumulation stays fp32 in PSUM.
            BF = mybir.dt.bfloat16
            VO = fr.tile([NB, NS, 3, 32], F)
            nc.vector.memset(VO[:, :, 2, :], 1.0)
            nc.vector.tensor_copy(VO[:, :, 0, :], D2P[:, :, :])
            nc.vector.tensor_copy(VO[:, :, 1, :], F0P[:, :, :])
            VV = fr.tile([NB, NS, 3, 32], BF)      # hi part
            nc.vector.tensor_copy(VV[:, :, :, :], VO[:, :, :, :])
            VHF = fr.tile([NB, NS, 3, 32], F)
            nc.vector.tensor_copy(VHF[:, :, :, :], VV[:, :, :, :])
            nc.vector.tensor_tensor(VHF[:, :, :, :], VO[:, :, :, :], VHF[:, :, :, :],
                                    A.subtract)    # reuse VHF as lo (f32)
            VL = fr.tile([NB, NS, 3, 32], BF)      # lo part
            nc.vector.tensor_copy(VL[:, :, :, :], VHF[:, :, :, :])

            # ---------------- Stage B: attn matmul streaming ----------------
            # attnT_loc[s] viewed [128, 32, 512]: row (p, j) = f = 32p + j.
            # SWDGE DMA casts f32 -> bf16 inline (HBM reads stay fp32).
            E = fr.tile([3, NS, TP], F)   # rows: curv_num, mean_num, S
            for s in range(NS):
                psE = psp.tile([3, TP], F, tag="psE")
                attv = attnT_loc[s, :, :].rearrange("(p j) c -> p j c", j=32)
                for i in range(4):
                    att = at.tile([NB, 8, TP], BF, tag="att")
                    nc.gpsimd.dma_start(att[:, :, :], attv[:, 8 * i:8 * (i + 1), :])
                    for jj in range(8):
                        j = 8 * i + jj
                        nc.tensor.matmul(psE[:, :], VV[:, s, :, j], att[:, jj, :],
                                         start=(j == 0), stop=False)
                        nc.tensor.matmul(psE[:, :], VL[:, s, :, j], att[:, jj, :],
                                         start=False, stop=(j == 31))
                nc.vector.tensor_copy(E[:, s, :], psE[:, :])

            # ---------------- Stage C: post-processing ----------------
            I3 = fr.tile([3, 3], F)
            nc.sync.dma_start(I3[:, :], i3_c[:, :])
            PP = fr.tile([NB, NS, PCH, 3], F)
            for s in range(NS):
                for c4 in range(PCH):
                    tp_ps = psp.tile([NB, 3], F, tag="tp")
                    nc.tensor.transpose(tp_ps[:, :],
                                        E[:, s, 128 * c4:128 * (c4 + 1)], I3[:, :])
                    nc.vector.tensor_copy(PP[:, s, c4, :], tp_ps[:, :])

            DEN = fr.tile([NB, NS, PCH], F)
            nc.vector.tensor_scalar(DEN[:, :, :], PP[:, :, :, 2], 2.0, 1.0,
                                    A.mult, A.max)
            nc.vector.tensor_scalar(DEN[:, :, :], DEN[:, :, :], EPS, None, A.add)
            RINV = fr.tile([NB, NS, PCH], F)
            nc.vector.reciprocal(RINV[:, :, :], DEN[:, :, :])
            CP = fr.tile([NB, NS, PCH], F)
            MPH = fr.tile([NB, NS, PCH], F)
            nc.vector.tensor_tensor(CP[:, :, :], PP[:, :, :, 0], RINV[:, :, :], A.mult)
            nc.vector.tensor_tensor(MPH[:, :, :], PP[:, :, :, 1], RINV[:, :, :], A.mult)
            nc.sync.dma_start(curv_out[:, :].rearrange("s (c k) -> k s c", k=NB),
                              CP[:, :, :])
            nc.sync.dma_start(mean_out[:, :].rearrange("s (c k) -> k s c", k=NB),
                              MPH[:, :, :])

            # masked semitone-deficit energies
            UVt = fr.tile([NB, NS, PCH], F)
            nc.sync.dma_start(UVt[:, :, :],
                              uv_loc[:, :].rearrange("s (c k) -> k s c", k=NB))
            SUMS = fr.tile([NB, 4, NS, PCH], F)  # kinds: def_mean, def_curv, val_mean, val_curv
            PVt = fr.tile([NB, NS, PCH], F)
            LNv = fr.tile([NB, NS, PCH], F)
            for m, (SRC, tgt) in enumerate([(MPH, MU_STAR), (CP, KAPPA_STAR)]):
                ood = 12.0 * float(np.log2(max(tgt, 1e-6) / 100.0))
                nc.vector.tensor_tensor(PVt[:, :, :], SRC[:, :, :], UVt[:, :, :],
                                        A.mult)
                nc.vector.tensor_scalar(SUMS[:, 2 + m, :, :], PVt[:, :, :], 0.0,
                                        None, A.not_equal)
                nc.vector.tensor_scalar(PVt[:, :, :], PVt[:, :, :], 1e-6, None, A.max)
                nc.scalar.activation(LNv[:, :, :], PVt[:, :, :], ACT.Ln,
                                     bias=0.0, scale=0.01)
                nc.vector.tensor_scalar(LNv[:, :, :], LNv[:, :, :], -12.0 / LN2,
                                        ood, A.mult, A.add)
                nc.vector.tensor_scalar(LNv[:, :, :], LNv[:, :, :], 0.0, None, A.max)
                nc.vector.tensor_tensor(SUMS[:, m, :, :], LNv[:, :, :],
                                        SUMS[:, 2 + m, :, :], A.mult)

            ONES = fr.tile([NB, 1], F)
            nc.vector.memset(ONES[:, :], 1.0)
            psR = psp1.tile([1, 32], F)
            nc.tensor.matmul(psR[:, :], ONES[:, :],
                             SUMS[:, :, :, :].rearrange("p a b c -> p (a b c)"),
                             start=True, stop=True)
            Rr = fr.tile([1, 4, NS, PCH], F)
            nc.vector.tensor_copy(Rr[:, :, :, :],
                                  psR[:, :].rearrange("p (a b c) -> p a b c",
                                                      a=4, b=NS))
            RSUM = fr.tile([1, 4, NS], F)
            nc.vector.tensor_reduce(RSUM[:, :, :], Rr[:, :, :, :], AX.X, A.add)
            VCNT = fr.tile([1, 2, NS], F)
            nc.vector.tensor_scalar(VCNT[:, :, :], RSUM[:, 2:4, :], 1.0, None, A.max)
            RCN = fr.tile([1, 2, NS], F)
            nc.vector.reciprocal(RCN[:, :, :], VCNT[:, :, :])
            TRM = fr.tile([1, 2, NS], F)
            nc.vector.tensor_tensor(TRM[:, :, :], RSUM[:, 0:2, :], RCN[:, :, :],
                                    A.mult)
            E2 = fr.tile([1, NS], F)
            nc.vector.tensor_tensor(E2[:, :], TRM[:, 0, :], TRM[:, 1, :], A.add)

            # ---------------- Stage D: AllGather + z-score softmax ----------------
            EA = fr.tile([1, B], F)
            if variant in ("full",):
                cin = dr.tile([1, NS], F)
                cout = dr.tile([NCORES, NS], F)
                nc.sync.dma_start(cin[:, :], E2[:, :])
                nc.gpsimd.collective_compute(
                    "AllGather", A.bypass,
                    replica_groups=[list(range(NCORES))],
                    ins=[cin[:, :].opt()],
                    outs=[cout[:, :].opt()],
                )
                nc.sync.dma_start(EA[0:1, :], cout[:, :].rearrange("r j -> (r j)"))
            else:
                nc.vector.memset(EA[0:1, :], 1.0)
                nc.vector.tensor_copy(EA[0:1, 0:NS], E2[:, :])

            MU1 = fr.tile([1, 1], F)
            nc.vector.tensor_reduce(MU1[:, :], EA[:, :], AX.X, A.add)
            nc.vector.tensor_scalar(MU1[:, :], MU1[:, :], 1.0 / B, None, A.mult)
            DD = fr.tile([1, B], F)
            nc.vector.tensor_scalar(DD[:, :], EA[:, :], MU1[:, 0:1], None, A.subtract)
            SQ = fr.tile([1, B], F)
            nc.vector.tensor_tensor(SQ[:, :], DD[:, :], DD[:, :], A.mult)
            VAR = fr.tile([1, 1], F)
            nc.vector.tensor_reduce(VAR[:, :], SQ[:, :], AX.X, A.add)
            nc.vector.tensor_scalar(VAR[:, :], VAR[:, :], 1.0 / B, None, A.mult)
            LV1 = fr.tile([1, 1], F)
            nc.scalar.activation(LV1[:, :], VAR[:, :], ACT.Ln)
            RSD = fr.tile([1, 1], F)
            nc.scalar.activation(RSD[:, :], LV1[:, :], ACT.Exp, bias=0.0, scale=-0.5)
            nc.vector.tensor_scalar(RSD[:, :], RSD[:, :], 1e6, None, A.min)
            Z = fr.tile([1, B], F)
            nc.vector.tensor_scalar(Z[:, :], DD[:, :], RSD[:, 0:1], None, A.mult)
            EX = fr.tile([1, B], F)
            nc.scalar.activation(EX[:, :], Z[:, :], ACT.Exp, bias=0.0, scale=-BETA)
            ES = fr.tile([1, 1], F)
            nc.vector.tensor_reduce(ES[:, :], EX[:, :], AX.X, A.add)
            RES = fr.tile([1, 1], F)
            nc.vector.reciprocal(RES[:, :], ES[:, :])
            WTS = fr.tile([1, B], F)
            nc.vector.tensor_scalar(WTS[:, :], EX[:, :], RES[:, 0:1], None, A.mult)
            W16 = fr.tile([1, B], F)
            nc.vector.tensor_scalar(W16[:, :], WTS[:, :], float(B), None, A.mult)
            nc.sync.dma_start(wts_out[:, :], W16[:, :])
            SQW = fr.tile([1, B], F)
            nc.vector.tensor_tensor(SQW[:, :], WTS[:, :], WTS[:, :], A.mult)
            SSW = fr.tile([1, 1], F)
            nc.vector.tensor_reduce(SSW[:, :], SQW[:, :], AX.X, A.add)
            ESS = fr.tile([1, 1], F)
            nc.vector.reciprocal(ESS[:, :], SSW[:, :])
            nc.sync.dma_start(ess_out[:, :], ESS[:, :])

    nc.compile()
    return nc


def get_program(variant="full"):
    if variant not in _PROG_CACHE:
        _PROG_CACHE[variant] = _build_program(variant)
    return _PROG_CACHE[variant]


def make_in_maps(f0, attnT, uv):
    maps = []
    for i in range(NCORES):
        s0 = NS * i
        maps.append({
            "f0_loc": np.ascontiguousarray(f0[s0:s0 + NS], np.float32),
            "attnT_loc": np.ascontiguousarray(attnT[s0:s0 + NS], np.float32),
            "uv_loc": np.ascontiguousarray(uv[s0:s0 + NS], np.float32),
        })
    return maps


def assemble(results):
    curv = np.concatenate([r["curv_out"] for r in results], axis=0)
    mean = np.concatenate([r["mean_out"] for r in results], axis=0)
    wts = np.asarray(results[0]["wts_out"], np.float32).reshape(B)
    ess = np.float32(np.asarray(results[0]["ess_out"]).reshape(()))
    return curv.astype(np.float32), mean.astype(np.float32), wts, ess


def kernel(f0_gd_frame, f2p_attn_gd, uv_mask):
    from concourse.bass_utils import run_bass_kernel_spmd

    f0 = np.asarray(f0_gd_frame, np.float32)
    attn = np.asarray(f2p_attn_gd, np.float32)
    uv = np.asarray(uv_mask, np.float32)
    attnT = np.ascontiguousarray(attn.transpose(0, 2, 1))  # [B, TF2, TP]

    nc = get_program()
    res = run_bass_kernel_spmd(nc, make_in_maps(f0, attnT, uv),
                               core_ids=list(range(NCORES)))
    return assemble(res.results)
